# revision 40
# baseline (speedup 1.0000x reference)
# DDSP synthesizer kernel for Trainium2 (8 NeuronCores, batch-parallel).
#
# Per core (one batch element):
#   harmonic branch: exact-phase oscillator bank. Phases are wrapped to
#     [-0.5,0.5] in (j,f) layout, split hi/lo bf16, PE-transposed to
#     time-major, then k*w computed as a K=2 bf16 outer-product matmul
#     (exact products, fp32 PSUM accumulate). Magic-number wrap
#     (ScalarE/DVE split), ScalarE Sin, per-frame weighted reduction on PE.
#   noise branch: irfft+window folded into one DFT matmul, frame-wise
#     128-tap causal conv via rfft-256 as PE matmuls (bf16 DFTs).
#   reverb: 16000-tap causal FIR as accumulating bf16 Toeplitz matmuls,
#     scheduled progressively: the harmonic signal is folded into the
#     padded signal tile every 4 chunks and each tap launches as soon as
#     its input window is complete, so there is no serial reverb tail.
#
# v2 layout/scheduling changes vs v1:
#   - packed const + param DRAM tensors (3 const DMAs, 4 param DMAs)
#   - contiguous reverb-noise load, impulse built directly in (block,
#     sample) layout (kills a 10us strided DMA + a transpose)
#   - phase scratch read back in 2 large DMAs into a resident SBUF tile
#   - noise-branch DFT matmuls in bf16, products on GpSimd
#   - scalar activation-table phases grouped: Exp -> Ln -> Exp -> Sin
#   - Toeplitz gather split by tap range so early taps unblock first
import math
import numpy as np

import concourse.bass as bass
import concourse.bacc as bacc
import concourse.mybir as mybir
from concourse import tile
from concourse.bass_utils import run_bass_kernel_spmd

F32 = mybir.dt.float32
F16 = mybir.dt.float16
BF16 = mybir.dt.bfloat16
B, F, NH, NB = 8, 400, 100, 65
SR, BLOCK = 16000, 128
T = F * BLOCK
LOG10 = math.log(10.0)
MAGIC = 12582912.0  # 1.5 * 2**23
NCHUNK = 50         # harmonic chunks of 1024 samples
CHW = 1024
# packed const layout (cols of c_pk [128, 898]):
#   0:128 eye | 128:256 dcos | 256:384 dsin | 384:512 icre | 512:640 icim
#   640:768 air (rows 0:65) | 768 ccol | 769 d128
#   770:898 rows 0:2 = kvneg (twice) | 898:1026 row 0 = nyq
#   1026:1154 row 0 = jrevT (128-j) | 1154:1282 row 0 = ones
CPK_COLS = 1282
# reverb output regions (columns of the 400-frame output)
R1W = 232
R2W = F - R1W

_cache = {}


def _host_constants():
    b = np.arange(NB)[:, None]
    m = np.arange(128)[None, :]
    w = np.where((b == 0) | (b == 64), 1.0, 2.0)
    Cmat = w / 128.0 * np.cos(2 * np.pi * b * m / 128.0)
    win2 = 0.5 + 0.5 * np.cos(2 * np.pi * np.arange(128) / 128.0)
    Air = (2.0 * Cmat * win2[None, :]).astype(np.float32)               # (65,128)
    ccol = (1e-7 * (Cmat * win2[None, :]).sum(0)).astype(np.float32)    # (128,)
    j = np.arange(128)[:, None]
    bb = np.arange(128)[None, :]
    Dcos = np.cos(2 * np.pi * j * bb / 256.0).astype(np.float32)        # (128j,128b)
    Dsin = (-np.sin(2 * np.pi * j * bb / 256.0)).astype(np.float32)
    d128 = np.cos(np.pi * np.arange(128)).astype(np.float32)            # (128,)
    bb2 = np.arange(128)[:, None]
    i = np.arange(128)[None, :]
    cb = np.where(bb2 == 0, 1.0, 2.0)
    ICre = (cb / 256.0 * np.cos(2 * np.pi * bb2 * i / 256.0)).astype(np.float32)
    ICim = (-2.0 / 256.0 * np.sin(2 * np.pi * bb2 * i / 256.0)).astype(np.float32)
    nyq = ((1.0 / 256.0) * np.cos(np.pi * np.arange(128))).astype(np.float32)
    # output samples are produced block-reversed (partition p = sample 127-p)
    ICre = np.ascontiguousarray(ICre[:, ::-1])
    ICim = np.ascontiguousarray(ICim[:, ::-1])
    nyq = np.ascontiguousarray(nyq[::-1])
    kvneg = np.zeros(128, np.float32)
    kvneg[:NH] = -np.arange(1, NH + 1)
    eye = np.eye(128, dtype=np.float32)

    pk = np.zeros((128, CPK_COLS), np.float32)
    pk[:, 0:128] = eye
    pk[:, 128:256] = Dcos
    pk[:, 256:384] = Dsin
    pk[:, 384:512] = ICre
    pk[:, 512:640] = ICim
    pk[0:NB, 640:768] = Air
    pk[:, 768] = ccol
    pk[:, 769] = d128
    pk[0, 770:898] = kvneg
    pk[1, 770:898] = kvneg
    pk[0, 898:1026] = nyq
    pk[0, 1026:1154] = 128.0 - np.arange(128, dtype=np.float32)
    pk[0, 1154:1282] = 1.0
    return dict(c_pk=np.ascontiguousarray(pk))


def _build():
    nc = bacc.Bacc(None, target_bir_lowering=False, debug=False)

    pit_d = nc.dram_tensor("pitchpk", [F + 2, 1], F32, kind="ExternalInput")
    par_d = nc.dram_tensor("parampk", [F, 294], F32, kind="ExternalInput")
    rvn_d = nc.dram_tensor("reverb_noise", [SR, 1], F32, kind="ExternalInput")
    cpk_d = nc.dram_tensor("c_pk", [128, CPK_COLS], F32, kind="ExternalInput")
    out_d = nc.dram_tensor("out", [128, F], F32, kind="ExternalOutput")

    wscr16 = nc.dram_tensor("wscr16", [2, T], BF16)
    hpbuf16 = nc.dram_tensor("hpbuf16", [1, 16384], BF16)

    AF = mybir.ActivationFunctionType
    OP = mybir.AluOpType

    with tile.TileContext(nc) as tc:
        with tc.tile_pool(name="const", bufs=1) as cpool, \
             tc.tile_pool(name="big", bufs=1) as big, \
             tc.tile_pool(name="work", bufs=1) as work, \
             tc.tile_pool(name="chunk", bufs=2) as chk:

            # ---------- earliest DMAs, spread across queues ----------
            # sync: pitch (critical path) then pc4 then params
            t_pr2 = work.tile([1, F + 2], F32)
            nc.sync.dma_start(t_pr2[:], bass.AP(pit_d, 0, [[1, 1], [1, F + 2]]))
            t_pr = t_pr2[:, 0:F]
            # sync queue: eye first (transposes), then nyq/kv rows
            t_pk = big.tile([128, CPK_COLS], F32)
            nc.sync.dma_start(t_pk[:, 0:128], cpk_d[:, 0:128])
            nc.sync.dma_start(t_pk[0:2, 770:1282],
                              bass.AP(cpk_d, 770, [[CPK_COLS, 2], [1, 512]]))
            # scalar queue: the rest of the consts
            nc.scalar.dma_start(t_pk[:, 128:770],
                                bass.AP(cpk_d, 128, [[CPK_COLS, 128], [1, 642]]))
            # gpsimd: reverb noise, contiguous (block, sample) layout
            t_rn2 = work.tile([125, 128], F32)
            nc.gpsimd.dma_start(t_rn2[:], bass.AP(rvn_d, 0, [[128, 125], [1, 128]]))
            t_eye = t_pk[:, 0:128]

            # ---------- small SBUF constants ----------
            t_mcol = cpool.tile([128, 1], F32)
            nc.vector.memset(t_mcol[:], MAGIC)
            t_b5 = cpool.tile([128, 1], F32)
            nc.vector.memset(t_b5[:], 5.0)
            t_eps = cpool.tile([128, 1], F32)
            nc.vector.memset(t_eps[:], 1e-7)
            t_kroi = cpool.tile([128, NH], mybir.dt.int32)
            nc.gpsimd.iota(t_kroi[:], pattern=[[1, NH]], base=1, channel_multiplier=0)
            t_krow = cpool.tile([128, NH], F32)
            nc.gpsimd.tensor_copy(t_krow[:], t_kroi[:])

            # impulse iota in (block, sample) layout: val = 128*b + s
            t_ioti2 = cpool.tile([125, 128], mybir.dt.int32)
            nc.gpsimd.iota(t_ioti2[:], pattern=[[1, 128]], base=0,
                           channel_multiplier=128)
            t_iotf2 = cpool.tile([125, 128], F32)
            nc.gpsimd.tensor_copy(t_iotf2[:], t_ioti2[:])
            # bf16 copies of PE constants (gpsimd; SBUF only)
            t_eye16 = cpool.tile([128, 128], BF16)
            nc.gpsimd.tensor_copy(t_eye16[:], t_eye)
            t_kv16 = cpool.tile([2, 128], BF16)
            nc.gpsimd.tensor_copy(t_kv16[:], t_pk[0:2, 770:898])
            t_dcos16 = cpool.tile([128, 128], BF16)
            nc.gpsimd.tensor_copy(t_dcos16[:], t_pk[:, 128:256])
            t_dsin16 = cpool.tile([128, 128], BF16)
            nc.gpsimd.tensor_copy(t_dsin16[:], t_pk[:, 256:384])
            t_d12816 = cpool.tile([128, 1], BF16)
            nc.gpsimd.tensor_copy(t_d12816[:], t_pk[:, 769:770])
            t_air16 = cpool.tile([NB, 128], BF16)
            nc.gpsimd.tensor_copy(t_air16[:], t_pk[0:NB, 640:768])

            with tc.high_priority():
                # ---------- phase chain (critical path to chunk 0) ----------
                t_p8 = work.tile([1, F], F32, tag="pp", bufs=5)
                nc.vector.tensor_scalar(out=t_p8[:], in0=t_pr, scalar1=8.0,
                                        scalar2=MAGIC, op0=OP.mult, op1=OP.add)
                t_ph = work.tile([1, F], F32, tag="pp", bufs=5)
                nc.vector.tensor_scalar(out=t_ph[:], in0=t_p8[:], scalar1=MAGIC,
                                        scalar2=0.125, op0=OP.subtract, op1=OP.mult)
                t_pl = work.tile([1, F], F32, tag="pp", bufs=5)
                nc.vector.tensor_sub(t_pl[:], t_pr, t_ph[:])
                t_zr = work.tile([1, F], F32, tag="pp", bufs=5)
                nc.vector.memset(t_zr[:], 0.0)
                t_sh = work.tile([1, F + 1], F32, tag="ps", bufs=2)
                nc.vector.memset(t_sh[:, 0:1], 0.0)
                nc.vector.tensor_tensor_scan(t_sh[:, 1:F + 1], t_ph[:], t_zr[:], 0.0,
                                             OP.add, OP.add)
                t_sl = work.tile([1, F + 1], F32, tag="ps", bufs=2)
                nc.vector.memset(t_sl[:, 0:1], 0.0)
                nc.vector.tensor_tensor_scan(t_sl[:, 1:F + 1], t_pl[:], t_zr[:], 0.0,
                                             OP.add, OP.add)
                t_ts = work.tile([1, F], F32, tag="pp", bufs=5)
                nc.vector.tensor_add(t_ts[:], t_sh[:, 0:F], t_sl[:, 0:F])
                t_t2r = work.tile([1, F], F32, tag="pp", bufs=5)
                nc.vector.tensor_scalar(out=t_t2r[:], in0=t_ts[:], scalar1=1.0 / 125.0,
                                        scalar2=MAGIC, op0=OP.mult, op1=OP.add)
                t_n125 = work.tile([1, F], F32, tag="pp", bufs=5)
                nc.vector.tensor_scalar(out=t_n125[:], in0=t_t2r[:], scalar1=MAGIC,
                                        scalar2=None, op0=OP.subtract)
                t_u = work.tile([1, F], F32, tag="pp", bufs=5)
                nc.vector.scalar_tensor_tensor(out=t_u[:], in0=t_n125[:], scalar=-125.0,
                                               in1=t_sh[:, 0:F], op0=OP.mult, op1=OP.add)
                t_r125 = work.tile([1, F], F32, tag="pp", bufs=5)
                nc.vector.tensor_add(t_r125[:], t_u[:], t_sl[:, 0:F])
                t_om = work.tile([1, F], F32, tag="pp", bufs=5)
                nc.vector.tensor_scalar(out=t_om[:], in0=t_r125[:], scalar1=1.0 / 125.0,
                                        scalar2=None, op0=OP.mult)
                t_cr = work.tile([1, F], F32, tag="pp", bufs=5)
                nc.vector.tensor_scalar(out=t_cr[:], in0=t_pr, scalar1=1.0 / 16000.0,
                                        scalar2=None, op0=OP.mult)
                # wraw[j, f] = (128-j)*cr[f] + om[f] via two outer-product
                # matmuls straight into PSUM (saves 3 serial broadcast hops)
                with tc.tile_pool(name="pswr", bufs=1, space="PSUM") as pswr:
                    p_wraw = pswr.tile([128, F], F32)
                    nc.tensor.matmul(p_wraw[:], t_pk[0:1, 1026:1154], t_cr[:],
                                     start=True, stop=False)
                    nc.tensor.matmul(p_wraw[:], t_pk[0:1, 1154:1282], t_om[:],
                                     start=False, stop=True)
                    t_tw = work.tile([128, F], F32, tag="pw", bufs=3)
                    nc.vector.tensor_scalar(out=t_tw[:], in0=p_wraw[:], scalar1=MAGIC,
                                            scalar2=None, op0=OP.add)
                    t_wneg = work.tile([128, F], F32, tag="pw", bufs=3)
                    nc.vector.scalar_tensor_tensor(out=t_wneg[:], in0=t_tw[:],
                                                   scalar=MAGIC, in1=p_wraw[:],
                                                   op0=OP.subtract, op1=OP.subtract)
                t_whi16 = work.tile([128, F], BF16)
                nc.vector.tensor_copy(t_whi16[:], t_wneg[:])
                t_wlo16 = work.tile([128, F], BF16)
                nc.vector.tensor_sub(t_wlo16[:], t_wneg[:], t_whi16[:])
                with tc.tile_pool(name="pswt", bufs=2, space="PSUM") as pswt:
                    t_wThi = work.tile([100, 512], BF16)
                    t_wTlo = work.tile([100, 512], BF16)
                    for g in range(4):
                        p_th = pswt.tile([100, 128], BF16, tag="wt")
                        nc.tensor.transpose(p_th[:], t_whi16[:, g * 100:(g + 1) * 100],
                                            t_eye16[:])
                        nc.scalar.copy(t_wThi[:, g * 128:(g + 1) * 128], p_th[:])
                        p_tl = pswt.tile([100, 128], BF16, tag="wt")
                        nc.tensor.transpose(p_tl[:], t_wlo16[:, g * 100:(g + 1) * 100],
                                            t_eye16[:])
                        nc.scalar.copy(t_wTlo[:, g * 128:(g + 1) * 128], p_tl[:])
                    # group 0 first (unblocks chunks 0-11), then groups 1-3;
                    # hi on sync queue, lo on gpsimd queue, in parallel
                    nc.sync.dma_start(
                        bass.AP(wscr16, 0, [[128, 100], [1, 128]]),
                        t_wThi[:, 0:128])
                    nc.gpsimd.dma_start(
                        bass.AP(wscr16, T, [[128, 100], [1, 128]]),
                        t_wTlo[:, 0:128])
                    nc.sync.dma_start(
                        bass.AP(wscr16, 12800, [[128, 100], [12800, 3], [1, 128]]),
                        t_wThi[:, 128:512])
                    nc.gpsimd.dma_start(
                        bass.AP(wscr16, T + 12800, [[128, 100], [12800, 3], [1, 128]]),
                        t_wTlo[:, 128:512])
                # resident phase tile: 2 big read-backs (hi row 0, lo row 1)
                t_wall = big.tile([2, T], BF16)
                nc.sync.dma_start(t_wall[:, 0:12288],
                                  bass.AP(wscr16, 0, [[T, 2], [1, 12288]]))
                nc.sync.dma_start(t_wall[:, 12288:T],
                                  bass.AP(wscr16, 12288, [[T, 2], [1, T - 12288]]))

            # ---------- remaining input DMAs ----------
            t_pc4 = work.tile([4, 100], F32)
            nc.sync.dma_start(t_pc4[:], bass.AP(pit_d, 0, [[100, 4], [1, 100]]))
            par_g = []
            for g in range(4):
                t_pg = work.tile([100, 294], F32, tag=f"parg{g}", name=f"parg{g}")
                nc.sync.dma_start(t_pg[:], par_d[g * 100:(g + 1) * 100, :])
                par_g.append(t_pg)

            # ---------- impulse response (block, sample layout) ----------
            t_dec = t_pr2[:, F:F + 1]
            t_wet = t_pr2[:, F + 1:F + 2]
            t_e1 = work.tile([1, 1], F32)
            nc.scalar.activation(t_e1[:], t_dec, AF.Exp, scale=-1.0)
            t_ew = work.tile([1, 1], F32)
            nc.scalar.activation(t_ew[:], t_wet, AF.Exp, scale=-1.0)

            # ---------- scale functions: Exp phase ----------
            amp_sf = []
            ns_sf = []
            for g in range(4):
                t_sf = work.tile([100, NH + 1], F32, tag=f"ampsf{g}", name=f"ampsf{g}")
                nc.scalar.activation(t_sf[:], par_g[g][:, 0:NH + 1], AF.Exp, scale=-1.0)
                amp_sf.append(t_sf)
            for g in range(4):
                t_ns = work.tile([100, NB], F32, tag=f"nsf{g}", name=f"nsf{g}")
                nc.scalar.activation(t_ns[:], par_g[g][:, NH + 1:NH + 1 + NB], AF.Exp,
                                     bias=t_b5[0:100, :], scale=-1.0)
                ns_sf.append(t_ns)
            # ---------- Ln phase ----------
            t_sp = work.tile([1, 1], F32)
            nc.scalar.activation(t_sp[:], t_e1[:], AF.Ln, bias=1.0, scale=1.0)
            for t_sf in amp_sf + ns_sf:
                nc.scalar.activation(t_sf[:], t_sf[:], AF.Ln, bias=1.0, scale=1.0)
            # ---------- Exp phase 2 (incl. reverb envelope) ----------
            for t_sf in amp_sf + ns_sf:
                nc.scalar.activation(t_sf[:], t_sf[:], AF.Exp, scale=-LOG10)
            t_s32 = work.tile([1, 1], F32)
            nc.vector.tensor_scalar(out=t_s32[:], in0=t_sp[:], scalar1=-1.0 / 32.0,
                                    scalar2=None, op0=OP.mult)
            t_ew1 = work.tile([1, 1], F32)
            nc.vector.tensor_scalar(out=t_ew1[:], in0=t_ew[:], scalar1=1.0,
                                    scalar2=None, op0=OP.add)
            t_sw = work.tile([1, 1], F32)
            nc.vector.reciprocal(t_sw[:], t_ew1[:])
            t_s32b = work.tile([128, 1], F32)
            nc.gpsimd.partition_broadcast(t_s32b[:], t_s32[:])
            t_swb = work.tile([128, 1], F32)
            nc.gpsimd.partition_broadcast(t_swb[:], t_sw[:])
            t_env2 = work.tile([125, 128], F32)
            nc.scalar.activation(t_env2[:], t_iotf2[:], AF.Exp,
                                 scale=t_s32b[0:125, :])
            t_h2 = work.tile([125, 128], F32)
            nc.vector.scalar_tensor_tensor(out=t_h2[:], in0=t_env2[:],
                                           scalar=t_swb[0:125, :], in1=t_rn2[:],
                                           op0=OP.mult, op1=OP.mult)
            nc.vector.memset(t_h2[0:1, 0:1], 1.0)
            t_h16 = work.tile([125, 128], BF16)
            nc.gpsimd.tensor_copy(t_h16[:], t_h2[:])
            t_z16 = work.tile([1, 160], BF16)
            nc.gpsimd.memset(t_z16[:], 0.0)
            nc.gpsimd.dma_start(bass.AP(hpbuf16, 0, [[1, 1], [1, 127]]),
                                t_z16[0:1, 0:127])
            nc.gpsimd.dma_start(bass.AP(hpbuf16, 127, [[128, 125], [1, 128]]),
                                t_h16[:])
            nc.gpsimd.dma_start(bass.AP(hpbuf16, 16127, [[1, 1], [1, 129]]),
                                t_z16[0:1, 0:129])
            # Toeplitz gather, split by tap range (high taps needed first)
            t_hs16 = big.tile([128, 16128], BF16)
            for d0, nd in [(94, 32), (62, 32), (30, 32), (0, 30)]:
                nc.gpsimd.dma_start(
                    t_hs16[:, d0 * 128:(d0 + nd) * 128],
                    bass.AP(hpbuf16, d0 * 128, [[1, 128], [128, nd], [1, 128]]))

            # amp post-scale (x2 + eps) on vector (keeps scalar free)
            for t_sf in amp_sf:
                nc.vector.tensor_scalar(out=t_sf[:], in0=t_sf[:], scalar1=2.0,
                                        scalar2=1e-7, op0=OP.mult, op1=OP.add)

            with tc.tile_pool(name="pstr", bufs=2, space="PSUM") as pstr:
                # ---------- amp params ----------
                t_at16 = big.tile([128, F], F16)
                nc.vector.memset(t_at16[:], 0.0)
                p_pc = pstr.tile([100, 4], F32, tag="tr")
                nc.tensor.transpose(p_pc[:], t_pc4[:], t_eye[0:4, 0:4])
                t_pcol = work.tile([100, 4], F32)
                nc.vector.tensor_copy(t_pcol[:], p_pc[:])
                for g in range(4):
                    f0 = g * 100
                    t_sf = amp_sf[g]
                    t_kp = work.tile([100, NH], F32, tag="kp")
                    nc.vector.tensor_scalar(out=t_kp[:], in0=t_krow[0:100, :],
                                            scalar1=t_pcol[:, g:g + 1], scalar2=None,
                                            op0=OP.mult)
                    t_aa = work.tile([100, NH], F32, tag="aa")
                    nc.vector.tensor_scalar(out=t_aa[:], in0=t_kp[:], scalar1=8000.0,
                                            scalar2=1e-4, op0=OP.is_lt, op1=OP.add)
                    t_am = work.tile([100, NH], F32, tag="am")
                    nc.vector.tensor_mul(t_am[:], t_sf[:, 1:NH + 1], t_aa[:])
                    t_ssum = work.tile([100, 1], F32, tag="ssum")
                    nc.vector.tensor_reduce(out=t_ssum[:], in_=t_am[:],
                                            axis=mybir.AxisListType.X, op=OP.add)
                    t_rec = work.tile([100, 1], F32, tag="rec")
                    nc.vector.reciprocal(t_rec[:], t_ssum[:])
                    t_scn = work.tile([100, 1], F32, tag="scn")
                    nc.vector.tensor_mul(t_scn[:], t_rec[:], t_sf[:, 0:1])
                    nc.vector.tensor_scalar(out=t_scn[:], in0=t_scn[:], scalar1=-1.0,
                                            scalar2=None, op0=OP.mult)
                    nc.vector.tensor_scalar(out=t_am[:], in0=t_am[:],
                                            scalar1=t_scn[:, :], scalar2=None,
                                            op0=OP.mult)
                    p_tr = pstr.tile([100, 100], F32, tag="tr")
                    nc.tensor.transpose(p_tr[:], t_am[:], t_eye[0:100, 0:100])
                    nc.vector.tensor_copy(t_at16[0:100, f0:f0 + 100], p_tr[:])

                # ---------- noise branch fronthalf ----------
                t_nt = big.tile([65, F], BF16)
                for g in range(4):
                    f0 = g * 100
                    p_tr2 = pstr.tile([65, 100], F32, tag="tr")
                    nc.tensor.transpose(p_tr2[:], ns_sf[g][:], t_eye[0:100, 0:100])
                    nc.vector.tensor_copy(t_nt[0:65, f0:f0 + 100], p_tr2[:])
                t_noT = big.tile([128, F], BF16)
                for g in range(4):
                    t_nog = par_g[g][:, NH + 1 + NB:294]
                    p_not = pstr.tile([128, 100], F32, tag="tr")
                    nc.tensor.transpose(p_not[:], t_nog, t_eye[0:100, 0:100])
                    nc.vector.tensor_copy(t_noT[:, g * 100:(g + 1) * 100], p_not[:])

                with tc.tile_pool(name="psn", bufs=1, space="PSUM") as psn:
                    p_irp = psn.tile([128, F], F32, tag="tmp")
                    nc.tensor.matmul(p_irp[:], t_air16[:], t_nt[0:65, :],
                                     start=True, stop=True)
                    t_irp = big.tile([128, F], BF16)
                    nc.vector.tensor_scalar(out=t_irp[:], in0=p_irp[:],
                                            scalar1=t_pk[:, 768:769], scalar2=None,
                                            op0=OP.add)
                    p_hre = psn.tile([128, F], F32, tag="hre")
                    nc.tensor.matmul(p_hre[:], t_dcos16[:], t_irp[:], start=True, stop=True)
                    t_hre = big.tile([128, F], BF16)
                    nc.scalar.copy(t_hre[:], p_hre[:])
                    p_him = psn.tile([128, F], F32, tag="hre")
                    nc.tensor.matmul(p_him[:], t_dsin16[:], t_irp[:], start=True, stop=True)
                    t_him = big.tile([128, F], BF16)
                    nc.scalar.copy(t_him[:], p_him[:])
                    p_h128 = psn.tile([1, F], F32, tag="tmp")
                    nc.tensor.matmul(p_h128[:], t_d12816[:], t_irp[:], start=True, stop=True)
                    t_h128 = big.tile([1, F], BF16)
                    nc.scalar.copy(t_h128[:], p_h128[:])
                    p_nre = psn.tile([128, F], F32, tag="hre")
                    nc.tensor.matmul(p_nre[:], t_dcos16[:], t_noT[:], start=True, stop=True)
                    t_nre = big.tile([128, F], BF16)
                    nc.scalar.copy(t_nre[:], p_nre[:])
                    p_nim = psn.tile([128, F], F32, tag="hre")
                    nc.tensor.matmul(p_nim[:], t_dsin16[:], t_noT[:], start=True, stop=True)
                    t_nim = big.tile([128, F], BF16)
                    nc.scalar.copy(t_nim[:], p_nim[:])
                    p_n128 = psn.tile([1, F], F32, tag="tmp")
                    nc.tensor.matmul(p_n128[:], t_d12816[:], t_noT[:], start=True, stop=True)
                    t_n128 = big.tile([1, F], BF16)
                    nc.scalar.copy(t_n128[:], p_n128[:])

            # frequency-domain products on gpsimd (SBUF bf16, overlaps chunks)
            t_a = work.tile([128, F], BF16, tag="pa")
            nc.gpsimd.tensor_mul(t_a[:], t_hre[:], t_nre[:])
            t_bp = work.tile([128, F], BF16, tag="pb")
            nc.gpsimd.tensor_mul(t_bp[:], t_him[:], t_nim[:])
            t_pre = big.tile([128, F], BF16)
            nc.gpsimd.tensor_sub(t_pre[:], t_a[:], t_bp[:])
            t_c2 = work.tile([128, F], BF16, tag="pa")
            nc.gpsimd.tensor_mul(t_c2[:], t_him[:], t_nre[:])
            t_d2 = work.tile([128, F], BF16, tag="pb")
            nc.gpsimd.tensor_mul(t_d2[:], t_hre[:], t_nim[:])
            t_pim = big.tile([128, F], BF16)
            nc.gpsimd.tensor_add(t_pim[:], t_c2[:], t_d2[:])
            t_p128 = big.tile([1, F], BF16)
            nc.gpsimd.tensor_mul(t_p128[:], t_h128[:], t_n128[:])

            t_spad16 = big.tile([128, 526], BF16)
            nc.vector.memset(t_spad16[:, 0:126], 0.0)

            t_icre16 = cpool.tile([128, 128], BF16)
            nc.vector.tensor_copy(t_icre16[:], t_pk[:, 384:512])
            t_icim16 = cpool.tile([128, 128], BF16)
            nc.vector.tensor_copy(t_icim16[:], t_pk[:, 512:640])
            t_nyq16 = cpool.tile([1, 128], BF16)
            nc.vector.tensor_copy(t_nyq16[:], t_pk[0:1, 898:1026])

            # ---------- reverb tap schedule ----------
            folds = list(range(3, 48, 4)) + [49]
            regions = [(0, R1W), (R1W, R2W)]
            taps = []
            for ri, (off, wdt) in enumerate(regions):
                for d in range(125, -1, -1):
                    need = off + wdt - d  # blocks required in spad
                    rc = next(cj for cj in folds if 8 * (cj + 1) >= need)
                    taps.append((rc, ri, d))
            taps.sort(key=lambda t: (t[0], -t[2]))
            TAP_BUDGET = 9
            region_seen = [0, 0]
            region_total = [126, 126]

            # ---------- main loop ----------
            with tc.tile_pool(name="pskw", bufs=3, space="PSUM") as pskw, \
                 tc.tile_pool(name="psO", bufs=1, space="PSUM") as psO, \
                 tc.tile_pool(name="psrev", bufs=1, space="PSUM") as psr:
                p_O = psO.tile([128, F], F32)
                p_nz = psr.tile([128, F], F32, tag="rz")
                p_rev = None
                kws, t1s, svs, sns = {}, {}, {}, {}
                tap_idx = 0
                fold_ptr = 0
                folds_done = -1
                out_started = [False, False]

                for c in range(NCHUNK + 4):
                    if c < NCHUNK:
                        p_kw = pskw.tile([128, CHW], F32, tag="kw")
                        for half in range(2):
                            nc.tensor.matmul(
                                p_kw[:, half * 512:(half + 1) * 512],
                                t_kv16[:],
                                t_wall[:, c * CHW + half * 512:c * CHW + (half + 1) * 512],
                                start=True, stop=True)
                        kws[c] = p_kw
                    # noise-branch backhalf interleaved into early chunks
                    if c == 4:
                        nc.tensor.matmul(p_nz[:], t_icre16[:], t_pre[:],
                                         start=True, stop=False)
                        nc.tensor.matmul(p_nz[:], t_icim16[:], t_pim[:],
                                         start=False, stop=False)
                        nc.tensor.matmul(p_nz[:], t_nyq16[:], t_p128[:],
                                         start=False, stop=True)
                    if c == 6:
                        nc.scalar.copy(t_spad16[:, 126:526], p_nz[:])
                    if c == 7:
                        p_rev = psr.tile([128, F], F32, tag="rz")
                    if c - 1 >= 0 and c - 1 < NCHUNK:
                        cj = c - 1
                        pk = kws[cj]
                        t_t1 = chk.tile([128, CHW], F32, tag="t1")
                        if cj % 5 in (1, 3):
                            nc.vector.tensor_scalar(out=t_t1[:], in0=pk[:], scalar1=MAGIC,
                                                    scalar2=None, op0=OP.add)
                        else:
                            nc.scalar.activation(t_t1[:], pk[:], AF.Identity,
                                                 bias=t_mcol[:, :], scale=1.0)
                        t1s[cj] = t_t1
                    if c - 2 >= 0 and c - 2 < NCHUNK:
                        t_sv = chk.tile([128, CHW], F32, tag="sv")
                        nc.vector.scalar_tensor_tensor(out=t_sv[:], in0=t1s.pop(c - 2)[:],
                                                       scalar=MAGIC, in1=kws.pop(c - 2)[:],
                                                       op0=OP.subtract, op1=OP.subtract)
                        svs[c - 2] = t_sv
                    if c - 3 >= 0 and c - 3 < NCHUNK:
                        t_sn = chk.tile([128, CHW], F16, tag="sn")
                        nc.scalar.activation(t_sn[:], svs.pop(c - 3)[:], AF.Sin,
                                             scale=2.0 * math.pi)
                        sns[c - 3] = t_sn
                    if c - 4 >= 0:
                        cj = c - 4
                        t_sn = sns.pop(cj)
                        for fl in range(8):
                            f = 8 * cj + fl
                            nc.tensor.matmul(p_O[:, f:f + 1],
                                             t_sn[0:NH, fl * 128:(fl + 1) * 128],
                                             t_at16[0:NH, f:f + 1], start=True, stop=True)
                        if fold_ptr < len(folds) and folds[fold_ptr] == cj:
                            fa = 8 * (folds[fold_ptr - 1] + 1) if fold_ptr > 0 else 0
                            fb = 8 * (cj + 1)
                            nc.vector.tensor_add(t_spad16[:, 126 + fa:126 + fb],
                                                 t_spad16[:, 126 + fa:126 + fb],
                                                 p_O[:, fa:fb])
                            fold_ptr += 1
                            folds_done = cj
                        # launch ready reverb taps
                        nhere = 0
                        while (tap_idx < len(taps) and nhere < TAP_BUDGET
                               and taps[tap_idx][0] <= folds_done):
                            _, ri, d = taps[tap_idx]
                            off, wdt = regions[ri]
                            nc.tensor.matmul(
                                p_rev[:, off:off + wdt],
                                t_hs16[:, d * 128:(d + 1) * 128],
                                t_spad16[:, 126 - d + off:126 - d + off + wdt],
                                start=(region_seen[ri] == 0),
                                stop=(region_seen[ri] == region_total[ri] - 1))
                            region_seen[ri] += 1
                            tap_idx += 1
                            nhere += 1
                            if region_seen[ri] == region_total[ri] and not out_started[ri]:
                                out_started[ri] = True
                                t_outr = big.tile([128, wdt], F32, tag=f"out{ri}",
                                                  name=f"out{ri}")
                                nc.scalar.copy(t_outr[:], p_rev[:, off:off + wdt])
                                nc.sync.dma_start(
                                    bass.AP(out_d, off, [[F, 128], [1, wdt]]),
                                    t_outr[:])

                # drain remaining taps
                while tap_idx < len(taps):
                    _, ri, d = taps[tap_idx]
                    off, wdt = regions[ri]
                    nc.tensor.matmul(
                        p_rev[:, off:off + wdt],
                        t_hs16[:, d * 128:(d + 1) * 128],
                        t_spad16[:, 126 - d + off:126 - d + off + wdt],
                        start=(region_seen[ri] == 0),
                        stop=(region_seen[ri] == region_total[ri] - 1))
                    region_seen[ri] += 1
                    tap_idx += 1
                    if region_seen[ri] == region_total[ri] and not out_started[ri]:
                        out_started[ri] = True
                        t_outr = big.tile([128, wdt], F32, tag=f"out{ri}",
                                          name=f"out{ri}")
                        nc.scalar.copy(t_outr[:], p_rev[:, off:off + wdt])
                        nc.sync.dma_start(
                            bass.AP(out_d, off, [[F, 128], [1, wdt]]),
                            t_outr[:])

    nc.compile()
    return nc


def _prep_inputs(inputs):
    amp = np.ascontiguousarray(np.asarray(inputs["amp_param"], np.float32))
    npr = np.ascontiguousarray(np.asarray(inputs["noise_param"], np.float32))
    pit = np.ascontiguousarray(np.asarray(inputs["pitch"], np.float32))
    noi = np.ascontiguousarray(np.asarray(inputs["noise"], np.float32))
    rvn = np.ascontiguousarray(np.asarray(inputs["reverb_noise"], np.float32))
    dec = float(np.asarray(inputs["decay"]).reshape(-1)[0])
    wet = float(np.asarray(inputs["wet"]).reshape(-1)[0])
    consts = _cache["consts"]
    in_maps = []
    for b in range(B):
        parpk = np.ascontiguousarray(
            np.concatenate([amp[b], npr[b], noi[b]], axis=1).astype(np.float32))
        pitpk = np.concatenate(
            [pit[b].reshape(F, 1),
             np.array([[dec], [wet]], np.float32)], axis=0)
        m = dict(pitchpk=np.ascontiguousarray(pitpk), parampk=parpk,
                 reverb_noise=rvn)
        m.update(consts)
        in_maps.append(m)
    return in_maps


def kernel(**inputs):
    if "nc" not in _cache:
        _cache["consts"] = _host_constants()
        _cache["nc"] = _build()
    nc = _cache["nc"]
    in_maps = _prep_inputs(inputs)
    res = run_bass_kernel_spmd(nc, in_maps, list(range(B)))
    out = np.stack([res.results[b]["out"].T.reshape(T, 1) for b in range(B)])
    return out.astype(np.float32)


if __name__ == "__main__":
    rng = np.random.default_rng(0)
    ins = dict(
        amp_param=rng.standard_normal((B, F, NH + 1)).astype(np.float32),
        noise_param=rng.standard_normal((B, F, NB)).astype(np.float32),
        pitch=(rng.random((B, F, 1), np.float32) * 440 + 60),
        noise=(rng.random((B, F, BLOCK), np.float32) * 2 - 1),
        reverb_noise=(rng.random((SR, 1), np.float32) * 2 - 1),
        decay=np.ones(1, np.float32) * 5,
        wet=np.zeros(1, np.float32),
        sampling_rate=SR, block_size=BLOCK,
    )
    o = kernel(**ins)
    print("kernel out", o.shape, o.dtype, np.abs(o).max())


def _install_ntff_hook():
    import sys as _sys
    import types as _types
    try:
        import antenv.axon_hooks  # noqa: F401
        return
    except ImportError:
        pass
    from trn_agent_boot.trn_boot import _ntff_profile_via_ctypes
    hook = _ntff_profile_via_ctypes('/opt/axon/libaxon_pjrt.so')
    mod = _types.ModuleType('antenv.axon_hooks')
    _h = {'v': hook}
    mod.get_axon_ntff_profile_hook = lambda: _h['v']
    mod.set_axon_ntff_profile_hook = lambda h: _h.update(v=h)
    _sys.modules['antenv.axon_hooks'] = mod
    import antenv
    antenv.axon_hooks = mod


def run_timed(**inputs):
    """Re-run with NTFF tracing enabled; returns max per-core exec ns or None."""
    _install_ntff_hook()
    if "nc" not in _cache:
        _cache["consts"] = _host_constants()
        _cache["nc"] = _build()
    nc = _cache["nc"]
    in_maps = _prep_inputs(inputs)
    res = run_bass_kernel_spmd(nc, in_maps, list(range(B)), trace=True)
    if res.instructions_and_trace is not None:
        _cache["insts"] = res.instructions_and_trace[0]
    return res.exec_time_ns


# revision 42
# speedup vs baseline: 1.0797x; 1.0797x over previous
# DDSP synthesizer kernel for Trainium2 (8 NeuronCores, batch-parallel).
#
# Per core (one batch element):
#   harmonic branch: exact-phase oscillator bank. Phases are wrapped to
#     [-0.5,0.5] in (j,f) layout, split hi/lo bf16, PE-transposed to
#     time-major, then k*w computed as a K=2 bf16 outer-product matmul
#     (exact products, fp32 PSUM accumulate). Magic-number wrap
#     (ScalarE/DVE split), ScalarE Sin, per-frame weighted reduction on PE.
#   noise branch: irfft+window folded into one DFT matmul, frame-wise
#     128-tap causal conv via rfft-256 as PE matmuls (bf16 DFTs).
#   reverb: 16000-tap causal FIR as accumulating bf16 Toeplitz matmuls,
#     scheduled progressively: the harmonic signal is folded into the
#     padded signal tile every 4 chunks and each tap launches as soon as
#     its input window is complete, so there is no serial reverb tail.
#
# v2 layout/scheduling changes vs v1:
#   - packed const + param DRAM tensors (3 const DMAs, 4 param DMAs)
#   - contiguous reverb-noise load, impulse built directly in (block,
#     sample) layout (kills a 10us strided DMA + a transpose)
#   - phase scratch read back in 2 large DMAs into a resident SBUF tile
#   - noise-branch DFT matmuls in bf16, products on GpSimd
#   - scalar activation-table phases grouped: Exp -> Ln -> Exp -> Sin
#   - Toeplitz gather split by tap range so early taps unblock first
import math
import numpy as np

import concourse.bass as bass
import concourse.bacc as bacc
import concourse.mybir as mybir
from concourse import tile
from concourse.bass_utils import run_bass_kernel_spmd

F32 = mybir.dt.float32
F16 = mybir.dt.float16
BF16 = mybir.dt.bfloat16
B, F, NH, NB = 8, 400, 100, 65
SR, BLOCK = 16000, 128
T = F * BLOCK
LOG10 = math.log(10.0)
MAGIC = 12582912.0  # 1.5 * 2**23
NCHUNK = 50         # harmonic chunks of 1024 samples
CHW = 1024
# packed const layout (cols of c_pk [128, 898]):
#   0:128 eye | 128:256 dcos | 256:384 dsin | 384:512 icre | 512:640 icim
#   640:768 air (rows 0:65) | 768 ccol | 769 d128
#   770:898 rows 0:2 = kvneg (twice) | 898:1026 row 0 = nyq
#   1026:1154 row 0 = jrevT (128-j) | 1154:1282 row 0 = ones
CPK_COLS = 1282
# reverb output regions (columns of the 400-frame output)
R1W = 232
R2W = F - R1W

_cache = {}


def _host_constants():
    b = np.arange(NB)[:, None]
    m = np.arange(128)[None, :]
    w = np.where((b == 0) | (b == 64), 1.0, 2.0)
    Cmat = w / 128.0 * np.cos(2 * np.pi * b * m / 128.0)
    win2 = 0.5 + 0.5 * np.cos(2 * np.pi * np.arange(128) / 128.0)
    Air = (2.0 * Cmat * win2[None, :]).astype(np.float32)               # (65,128)
    ccol = (1e-7 * (Cmat * win2[None, :]).sum(0)).astype(np.float32)    # (128,)
    j = np.arange(128)[:, None]
    bb = np.arange(128)[None, :]
    Dcos = np.cos(2 * np.pi * j * bb / 256.0).astype(np.float32)        # (128j,128b)
    Dsin = (-np.sin(2 * np.pi * j * bb / 256.0)).astype(np.float32)
    d128 = np.cos(np.pi * np.arange(128)).astype(np.float32)            # (128,)
    bb2 = np.arange(128)[:, None]
    i = np.arange(128)[None, :]
    cb = np.where(bb2 == 0, 1.0, 2.0)
    ICre = (cb / 256.0 * np.cos(2 * np.pi * bb2 * i / 256.0)).astype(np.float32)
    ICim = (-2.0 / 256.0 * np.sin(2 * np.pi * bb2 * i / 256.0)).astype(np.float32)
    nyq = ((1.0 / 256.0) * np.cos(np.pi * np.arange(128))).astype(np.float32)
    # output samples are produced block-reversed (partition p = sample 127-p)
    ICre = np.ascontiguousarray(ICre[:, ::-1])
    ICim = np.ascontiguousarray(ICim[:, ::-1])
    nyq = np.ascontiguousarray(nyq[::-1])
    kvneg = np.zeros(128, np.float32)
    kvneg[:NH] = -np.arange(1, NH + 1)
    eye = np.eye(128, dtype=np.float32)

    pk = np.zeros((128, CPK_COLS), np.float32)
    pk[:, 0:128] = eye
    pk[:, 128:256] = Dcos
    pk[:, 256:384] = Dsin
    pk[:, 384:512] = ICre
    pk[:, 512:640] = ICim
    pk[0:NB, 640:768] = Air
    pk[:, 768] = ccol
    pk[:, 769] = d128
    pk[0, 770:898] = kvneg
    pk[1, 770:898] = kvneg
    pk[0, 898:1026] = nyq
    pk[0, 1026:1154] = 128.0 - np.arange(128, dtype=np.float32)
    pk[0, 1154:1282] = 1.0
    return dict(c_pk=np.ascontiguousarray(pk))


def _build():
    nc = bacc.Bacc(None, target_bir_lowering=False, debug=False)

    pit_d = nc.dram_tensor("pitchpk", [F + 2, 1], F32, kind="ExternalInput")
    par_d = nc.dram_tensor("parampk", [F, 294], F32, kind="ExternalInput")
    rvn_d = nc.dram_tensor("reverb_noise", [SR, 1], F32, kind="ExternalInput")
    cpk_d = nc.dram_tensor("c_pk", [128, CPK_COLS], F32, kind="ExternalInput")
    out_d = nc.dram_tensor("out", [128, F], F32, kind="ExternalOutput")

    wscr16 = nc.dram_tensor("wscr16", [2, T], BF16)
    hpbuf16 = nc.dram_tensor("hpbuf16", [1, 16384], BF16)

    AF = mybir.ActivationFunctionType
    OP = mybir.AluOpType

    with tile.TileContext(nc) as tc:
        with tc.tile_pool(name="const", bufs=1) as cpool, \
             tc.tile_pool(name="big", bufs=1) as big, \
             tc.tile_pool(name="work", bufs=1) as work, \
             tc.tile_pool(name="chunk", bufs=2) as chk:

            # ---------- earliest DMAs, spread across queues ----------
            # sync: pitch (critical path) then pc4 then params
            t_pr2 = work.tile([1, F + 2], F32)
            nc.sync.dma_start(t_pr2[:], bass.AP(pit_d, 0, [[1, 1], [1, F + 2]]))
            t_pr = t_pr2[:, 0:F]
            # sync queue: eye first (transposes), then nyq/kv rows
            t_pk = big.tile([128, CPK_COLS], F32)
            nc.sync.dma_start(t_pk[:, 0:128], cpk_d[:, 0:128])
            nc.sync.dma_start(t_pk[0:2, 770:1282],
                              bass.AP(cpk_d, 770, [[CPK_COLS, 2], [1, 512]]))
            # scalar queue: the rest of the consts
            nc.scalar.dma_start(t_pk[:, 128:770],
                                bass.AP(cpk_d, 128, [[CPK_COLS, 128], [1, 642]]))
            # gpsimd: reverb noise, contiguous (block, sample) layout
            t_rn2 = work.tile([125, 128], F32)
            nc.gpsimd.dma_start(t_rn2[:], bass.AP(rvn_d, 0, [[128, 125], [1, 128]]))
            t_eye = t_pk[:, 0:128]

            # ---------- small SBUF constants ----------
            t_mcol = cpool.tile([128, 1], F32)
            nc.vector.memset(t_mcol[:], MAGIC)
            t_b5 = cpool.tile([128, 1], F32)
            nc.vector.memset(t_b5[:], 5.0)
            t_eps = cpool.tile([128, 1], F32)
            nc.vector.memset(t_eps[:], 1e-7)
            t_kroi = cpool.tile([128, NH], mybir.dt.int32)
            nc.gpsimd.iota(t_kroi[:], pattern=[[1, NH]], base=1, channel_multiplier=0)
            t_krow = cpool.tile([128, NH], F32)
            nc.gpsimd.tensor_copy(t_krow[:], t_kroi[:])

            # impulse iota in (block, sample) layout: val = 128*b + s
            t_ioti2 = cpool.tile([125, 128], mybir.dt.int32)
            nc.gpsimd.iota(t_ioti2[:], pattern=[[1, 128]], base=0,
                           channel_multiplier=128)
            t_iotf2 = cpool.tile([125, 128], F32)
            nc.gpsimd.tensor_copy(t_iotf2[:], t_ioti2[:])
            # bf16 copies of PE constants (gpsimd; SBUF only)
            t_eye16 = cpool.tile([128, 128], BF16)
            nc.gpsimd.tensor_copy(t_eye16[:], t_eye)
            t_kv16 = cpool.tile([2, 128], BF16)
            nc.gpsimd.tensor_copy(t_kv16[:], t_pk[0:2, 770:898])
            t_dcos16 = cpool.tile([128, 128], BF16)
            nc.gpsimd.tensor_copy(t_dcos16[:], t_pk[:, 128:256])
            t_dsin16 = cpool.tile([128, 128], BF16)
            nc.gpsimd.tensor_copy(t_dsin16[:], t_pk[:, 256:384])
            t_d12816 = cpool.tile([128, 1], BF16)
            nc.gpsimd.tensor_copy(t_d12816[:], t_pk[:, 769:770])
            t_air16 = cpool.tile([NB, 128], BF16)
            nc.gpsimd.tensor_copy(t_air16[:], t_pk[0:NB, 640:768])

            with tc.high_priority():
                # ---------- phase chain (critical path to chunk 0) ----------
                t_p8 = work.tile([1, F], F32, tag="pp", bufs=5)
                nc.scalar.activation(t_p8[:], t_pr, AF.Identity,
                                     bias=t_mcol[0:1, :], scale=8.0)
                t_ph = work.tile([1, F], F32, tag="pp", bufs=5)
                nc.vector.tensor_scalar(out=t_ph[:], in0=t_p8[:], scalar1=MAGIC,
                                        scalar2=0.125, op0=OP.subtract, op1=OP.mult)
                t_pl = work.tile([1, F], F32, tag="pp", bufs=5)
                nc.vector.tensor_sub(t_pl[:], t_pr, t_ph[:])
                t_zr = work.tile([1, F], F32, tag="pp", bufs=5)
                nc.vector.memset(t_zr[:], 0.0)
                t_sh = work.tile([1, F + 1], F32, tag="ps", bufs=2)
                nc.vector.memset(t_sh[:, 0:1], 0.0)
                nc.vector.tensor_tensor_scan(t_sh[:, 1:F + 1], t_ph[:], t_zr[:], 0.0,
                                             OP.add, OP.add)
                t_sl = work.tile([1, F + 1], F32, tag="ps", bufs=2)
                nc.vector.memset(t_sl[:, 0:1], 0.0)
                nc.vector.tensor_tensor_scan(t_sl[:, 1:F + 1], t_pl[:], t_zr[:], 0.0,
                                             OP.add, OP.add)
                t_ts = work.tile([1, F], F32, tag="pp", bufs=5)
                nc.vector.tensor_add(t_ts[:], t_sh[:, 0:F], t_sl[:, 0:F])
                t_t2r = work.tile([1, F], F32, tag="pp", bufs=5)
                nc.scalar.activation(t_t2r[:], t_ts[:], AF.Identity,
                                     bias=t_mcol[0:1, :], scale=1.0 / 125.0)
                t_n125 = work.tile([1, F], F32, tag="pp", bufs=5)
                nc.vector.tensor_scalar(out=t_n125[:], in0=t_t2r[:], scalar1=MAGIC,
                                        scalar2=None, op0=OP.subtract)
                t_u = work.tile([1, F], F32, tag="pp", bufs=5)
                nc.vector.scalar_tensor_tensor(out=t_u[:], in0=t_n125[:], scalar=-125.0,
                                               in1=t_sh[:, 0:F], op0=OP.mult, op1=OP.add)
                t_r125 = work.tile([1, F], F32, tag="pp", bufs=5)
                nc.vector.tensor_add(t_r125[:], t_u[:], t_sl[:, 0:F])
                t_om = work.tile([1, F], F32, tag="pp", bufs=5)
                nc.vector.tensor_scalar(out=t_om[:], in0=t_r125[:], scalar1=1.0 / 125.0,
                                        scalar2=None, op0=OP.mult)
                t_cr = work.tile([1, F], F32, tag="pp", bufs=5)
                nc.vector.tensor_scalar(out=t_cr[:], in0=t_pr, scalar1=1.0 / 16000.0,
                                        scalar2=None, op0=OP.mult)
                # wraw[j, f] = (128-j)*cr[f] + om[f] via two outer-product
                # matmuls straight into PSUM (saves 3 serial broadcast hops)
                with tc.tile_pool(name="pswr", bufs=1, space="PSUM") as pswr:
                    p_wraw = pswr.tile([128, F], F32)
                    nc.tensor.matmul(p_wraw[:], t_pk[0:1, 1026:1154], t_cr[:],
                                     start=True, stop=False)
                    nc.tensor.matmul(p_wraw[:], t_pk[0:1, 1154:1282], t_om[:],
                                     start=False, stop=True)
                    t_tw = work.tile([128, F], F32, tag="pw", bufs=3)
                    nc.scalar.activation(t_tw[:], p_wraw[:], AF.Identity,
                                         bias=t_mcol[:, :], scale=1.0)
                    t_wneg = work.tile([128, F], F32, tag="pw", bufs=3)
                    nc.vector.scalar_tensor_tensor(out=t_wneg[:], in0=t_tw[:],
                                                   scalar=MAGIC, in1=p_wraw[:],
                                                   op0=OP.subtract, op1=OP.subtract)
                t_whi16 = work.tile([128, F], BF16)
                nc.vector.tensor_copy(t_whi16[:], t_wneg[:])
                t_wlo16 = work.tile([128, F], BF16)
                nc.vector.tensor_sub(t_wlo16[:], t_wneg[:], t_whi16[:])
                with tc.tile_pool(name="pswt", bufs=2, space="PSUM") as pswt:
                    t_wThi = work.tile([100, 512], BF16)
                    t_wTlo = work.tile([100, 512], BF16)
                    for g in range(4):
                        p_th = pswt.tile([100, 128], BF16, tag="wt")
                        nc.tensor.transpose(p_th[:], t_whi16[:, g * 100:(g + 1) * 100],
                                            t_eye16[:])
                        nc.scalar.copy(t_wThi[:, g * 128:(g + 1) * 128], p_th[:])
                        p_tl = pswt.tile([100, 128], BF16, tag="wt")
                        nc.tensor.transpose(p_tl[:], t_wlo16[:, g * 100:(g + 1) * 100],
                                            t_eye16[:])
                        nc.scalar.copy(t_wTlo[:, g * 128:(g + 1) * 128], p_tl[:])
                    # group 0 first (unblocks chunks 0-11), then groups 1-3;
                    # hi on sync queue, lo on gpsimd queue, in parallel
                    nc.sync.dma_start(
                        bass.AP(wscr16, 0, [[128, 100], [1, 128]]),
                        t_wThi[:, 0:128])
                    nc.gpsimd.dma_start(
                        bass.AP(wscr16, T, [[128, 100], [1, 128]]),
                        t_wTlo[:, 0:128])
                    nc.sync.dma_start(
                        bass.AP(wscr16, 12800, [[128, 100], [12800, 3], [1, 128]]),
                        t_wThi[:, 128:512])
                    nc.gpsimd.dma_start(
                        bass.AP(wscr16, T + 12800, [[128, 100], [12800, 3], [1, 128]]),
                        t_wTlo[:, 128:512])
                # resident phase tile: 2 big read-backs (hi row 0, lo row 1)
                t_wall = big.tile([2, T], BF16)
                nc.sync.dma_start(t_wall[:, 0:12288],
                                  bass.AP(wscr16, 0, [[T, 2], [1, 12288]]))
                nc.sync.dma_start(t_wall[:, 12288:T],
                                  bass.AP(wscr16, 12288, [[T, 2], [1, T - 12288]]))

            # ---------- remaining input DMAs ----------
            t_pc4 = work.tile([4, 100], F32)
            nc.sync.dma_start(t_pc4[:], bass.AP(pit_d, 0, [[100, 4], [1, 100]]))
            par_g = []
            for g in range(4):
                t_pg = work.tile([100, 294], F32, tag=f"parg{g}", name=f"parg{g}")
                nc.sync.dma_start(t_pg[:], par_d[g * 100:(g + 1) * 100, :])
                par_g.append(t_pg)

            # ---------- impulse response (block, sample layout) ----------
            t_dec = t_pr2[:, F:F + 1]
            t_wet = t_pr2[:, F + 1:F + 2]
            t_e1 = work.tile([1, 1], F32)
            nc.scalar.activation(t_e1[:], t_dec, AF.Exp, scale=-1.0)
            t_ew = work.tile([1, 1], F32)
            nc.scalar.activation(t_ew[:], t_wet, AF.Exp, scale=-1.0)

            # ---------- scale functions: Exp phase ----------
            amp_sf = []
            ns_sf = []
            for g in range(4):
                t_sf = work.tile([100, NH + 1], F32, tag=f"ampsf{g}", name=f"ampsf{g}")
                nc.scalar.activation(t_sf[:], par_g[g][:, 0:NH + 1], AF.Exp, scale=-1.0)
                amp_sf.append(t_sf)
            for g in range(4):
                t_ns = work.tile([100, NB], F32, tag=f"nsf{g}", name=f"nsf{g}")
                nc.scalar.activation(t_ns[:], par_g[g][:, NH + 1:NH + 1 + NB], AF.Exp,
                                     bias=t_b5[0:100, :], scale=-1.0)
                ns_sf.append(t_ns)
            # ---------- Ln phase ----------
            t_sp = work.tile([1, 1], F32)
            nc.scalar.activation(t_sp[:], t_e1[:], AF.Ln, bias=1.0, scale=1.0)
            for t_sf in amp_sf + ns_sf:
                nc.scalar.activation(t_sf[:], t_sf[:], AF.Ln, bias=1.0, scale=1.0)
            # ---------- Exp phase 2 (incl. reverb envelope) ----------
            for t_sf in amp_sf + ns_sf:
                nc.scalar.activation(t_sf[:], t_sf[:], AF.Exp, scale=-LOG10)
            t_s32 = work.tile([1, 1], F32)
            nc.vector.tensor_scalar(out=t_s32[:], in0=t_sp[:], scalar1=-1.0 / 32.0,
                                    scalar2=None, op0=OP.mult)
            t_ew1 = work.tile([1, 1], F32)
            nc.vector.tensor_scalar(out=t_ew1[:], in0=t_ew[:], scalar1=1.0,
                                    scalar2=None, op0=OP.add)
            t_sw = work.tile([1, 1], F32)
            nc.vector.reciprocal(t_sw[:], t_ew1[:])
            t_s32b = work.tile([128, 1], F32)
            nc.gpsimd.partition_broadcast(t_s32b[:], t_s32[:])
            t_swb = work.tile([128, 1], F32)
            nc.gpsimd.partition_broadcast(t_swb[:], t_sw[:])
            t_env2 = work.tile([125, 128], F32)
            nc.scalar.activation(t_env2[:], t_iotf2[:], AF.Exp,
                                 scale=t_s32b[0:125, :])
            t_h2 = work.tile([125, 128], F32)
            nc.vector.scalar_tensor_tensor(out=t_h2[:], in0=t_env2[:],
                                           scalar=t_swb[0:125, :], in1=t_rn2[:],
                                           op0=OP.mult, op1=OP.mult)
            nc.vector.memset(t_h2[0:1, 0:1], 1.0)
            t_h16 = work.tile([125, 128], BF16)
            nc.gpsimd.tensor_copy(t_h16[:], t_h2[:])
            t_z16 = work.tile([1, 160], BF16)
            nc.gpsimd.memset(t_z16[:], 0.0)
            nc.gpsimd.dma_start(bass.AP(hpbuf16, 0, [[1, 1], [1, 127]]),
                                t_z16[0:1, 0:127])
            nc.gpsimd.dma_start(bass.AP(hpbuf16, 127, [[128, 125], [1, 128]]),
                                t_h16[:])
            nc.gpsimd.dma_start(bass.AP(hpbuf16, 16127, [[1, 1], [1, 129]]),
                                t_z16[0:1, 0:129])
            # Toeplitz gather, split by tap range (high taps needed first)
            t_hs16 = big.tile([128, 16128], BF16)
            for d0, nd in [(94, 32), (62, 32), (30, 32), (0, 30)]:
                nc.gpsimd.dma_start(
                    t_hs16[:, d0 * 128:(d0 + nd) * 128],
                    bass.AP(hpbuf16, d0 * 128, [[1, 128], [128, nd], [1, 128]]))

            # amp post-scale (x2 + eps) on vector (keeps scalar free)
            for t_sf in amp_sf:
                nc.vector.tensor_scalar(out=t_sf[:], in0=t_sf[:], scalar1=2.0,
                                        scalar2=1e-7, op0=OP.mult, op1=OP.add)

            with tc.tile_pool(name="pstr", bufs=2, space="PSUM") as pstr:
                # ---------- amp params ----------
                t_at16 = big.tile([128, F], F16)
                nc.vector.memset(t_at16[:], 0.0)
                p_pc = pstr.tile([100, 4], F32, tag="tr")
                nc.tensor.transpose(p_pc[:], t_pc4[:], t_eye[0:4, 0:4])
                t_pcol = work.tile([100, 4], F32)
                nc.vector.tensor_copy(t_pcol[:], p_pc[:])
                for g in range(4):
                    f0 = g * 100
                    t_sf = amp_sf[g]
                    t_kp = work.tile([100, NH], F32, tag="kp")
                    nc.vector.tensor_scalar(out=t_kp[:], in0=t_krow[0:100, :],
                                            scalar1=t_pcol[:, g:g + 1], scalar2=None,
                                            op0=OP.mult)
                    t_aa = work.tile([100, NH], F32, tag="aa")
                    nc.vector.tensor_scalar(out=t_aa[:], in0=t_kp[:], scalar1=8000.0,
                                            scalar2=1e-4, op0=OP.is_lt, op1=OP.add)
                    t_am = work.tile([100, NH], F32, tag="am")
                    nc.vector.tensor_mul(t_am[:], t_sf[:, 1:NH + 1], t_aa[:])
                    t_ssum = work.tile([100, 1], F32, tag="ssum")
                    nc.vector.tensor_reduce(out=t_ssum[:], in_=t_am[:],
                                            axis=mybir.AxisListType.X, op=OP.add)
                    t_rec = work.tile([100, 1], F32, tag="rec")
                    nc.vector.reciprocal(t_rec[:], t_ssum[:])
                    t_scn = work.tile([100, 1], F32, tag="scn")
                    nc.vector.tensor_mul(t_scn[:], t_rec[:], t_sf[:, 0:1])
                    nc.vector.tensor_scalar(out=t_scn[:], in0=t_scn[:], scalar1=-1.0,
                                            scalar2=None, op0=OP.mult)
                    nc.vector.tensor_scalar(out=t_am[:], in0=t_am[:],
                                            scalar1=t_scn[:, :], scalar2=None,
                                            op0=OP.mult)
                    p_tr = pstr.tile([100, 100], F32, tag="tr")
                    nc.tensor.transpose(p_tr[:], t_am[:], t_eye[0:100, 0:100])
                    nc.vector.tensor_copy(t_at16[0:100, f0:f0 + 100], p_tr[:])

                # ---------- noise branch fronthalf ----------
                t_nt = big.tile([65, F], BF16)
                for g in range(4):
                    f0 = g * 100
                    p_tr2 = pstr.tile([65, 100], F32, tag="tr")
                    nc.tensor.transpose(p_tr2[:], ns_sf[g][:], t_eye[0:100, 0:100])
                    nc.vector.tensor_copy(t_nt[0:65, f0:f0 + 100], p_tr2[:])
                t_noT = big.tile([128, F], BF16)
                for g in range(4):
                    t_nog = par_g[g][:, NH + 1 + NB:294]
                    p_not = pstr.tile([128, 100], F32, tag="tr")
                    nc.tensor.transpose(p_not[:], t_nog, t_eye[0:100, 0:100])
                    nc.vector.tensor_copy(t_noT[:, g * 100:(g + 1) * 100], p_not[:])

                with tc.tile_pool(name="psn", bufs=1, space="PSUM") as psn:
                    p_irp = psn.tile([128, F], F32, tag="tmp")
                    nc.tensor.matmul(p_irp[:], t_air16[:], t_nt[0:65, :],
                                     start=True, stop=True)
                    t_irp = big.tile([128, F], BF16)
                    nc.vector.tensor_scalar(out=t_irp[:], in0=p_irp[:],
                                            scalar1=t_pk[:, 768:769], scalar2=None,
                                            op0=OP.add)
                    p_hre = psn.tile([128, F], F32, tag="hre")
                    nc.tensor.matmul(p_hre[:], t_dcos16[:], t_irp[:], start=True, stop=True)
                    t_hre = big.tile([128, F], BF16)
                    nc.scalar.copy(t_hre[:], p_hre[:])
                    p_him = psn.tile([128, F], F32, tag="hre")
                    nc.tensor.matmul(p_him[:], t_dsin16[:], t_irp[:], start=True, stop=True)
                    t_him = big.tile([128, F], BF16)
                    nc.scalar.copy(t_him[:], p_him[:])
                    p_h128 = psn.tile([1, F], F32, tag="tmp")
                    nc.tensor.matmul(p_h128[:], t_d12816[:], t_irp[:], start=True, stop=True)
                    t_h128 = big.tile([1, F], BF16)
                    nc.scalar.copy(t_h128[:], p_h128[:])
                    p_nre = psn.tile([128, F], F32, tag="hre")
                    nc.tensor.matmul(p_nre[:], t_dcos16[:], t_noT[:], start=True, stop=True)
                    t_nre = big.tile([128, F], BF16)
                    nc.scalar.copy(t_nre[:], p_nre[:])
                    p_nim = psn.tile([128, F], F32, tag="hre")
                    nc.tensor.matmul(p_nim[:], t_dsin16[:], t_noT[:], start=True, stop=True)
                    t_nim = big.tile([128, F], BF16)
                    nc.scalar.copy(t_nim[:], p_nim[:])
                    p_n128 = psn.tile([1, F], F32, tag="tmp")
                    nc.tensor.matmul(p_n128[:], t_d12816[:], t_noT[:], start=True, stop=True)
                    t_n128 = big.tile([1, F], BF16)
                    nc.scalar.copy(t_n128[:], p_n128[:])

            # frequency-domain products on gpsimd (SBUF bf16, overlaps chunks)
            t_a = work.tile([128, F], BF16, tag="pa")
            nc.gpsimd.tensor_mul(t_a[:], t_hre[:], t_nre[:])
            t_bp = work.tile([128, F], BF16, tag="pb")
            nc.gpsimd.tensor_mul(t_bp[:], t_him[:], t_nim[:])
            t_pre = big.tile([128, F], BF16)
            nc.gpsimd.tensor_sub(t_pre[:], t_a[:], t_bp[:])
            t_c2 = work.tile([128, F], BF16, tag="pa")
            nc.gpsimd.tensor_mul(t_c2[:], t_him[:], t_nre[:])
            t_d2 = work.tile([128, F], BF16, tag="pb")
            nc.gpsimd.tensor_mul(t_d2[:], t_hre[:], t_nim[:])
            t_pim = big.tile([128, F], BF16)
            nc.gpsimd.tensor_add(t_pim[:], t_c2[:], t_d2[:])
            t_p128 = big.tile([1, F], BF16)
            nc.gpsimd.tensor_mul(t_p128[:], t_h128[:], t_n128[:])

            t_spad16 = big.tile([128, 526], BF16)
            nc.vector.memset(t_spad16[:], 0.0)

            t_icre16 = cpool.tile([128, 128], BF16)
            nc.vector.tensor_copy(t_icre16[:], t_pk[:, 384:512])
            t_icim16 = cpool.tile([128, 128], BF16)
            nc.vector.tensor_copy(t_icim16[:], t_pk[:, 512:640])
            t_nyq16 = cpool.tile([1, 128], BF16)
            nc.vector.tensor_copy(t_nyq16[:], t_pk[0:1, 898:1026])

            # ---------- reverb tap schedule ----------
            folds = list(range(1, 16, 2)) + list(range(19, 48, 4)) + [49]
            regions = [(0, R1W), (R1W, R2W)]
            taps = []
            for ri, (off, wdt) in enumerate(regions):
                for d in range(125, -1, -1):
                    need = off + wdt - d  # blocks required in spad
                    rc = next(cj for cj in folds if 8 * (cj + 1) >= need)
                    taps.append((rc, ri, d))
            taps.sort(key=lambda t: (t[0], -t[2]))
            TAP_BUDGET = 9
            region_seen = [0, 0]
            region_total = [126, 126]

            # ---------- main loop ----------
            with tc.tile_pool(name="pskw", bufs=3, space="PSUM") as pskw, \
                 tc.tile_pool(name="psO", bufs=1, space="PSUM") as psO, \
                 tc.tile_pool(name="psrev", bufs=1, space="PSUM") as psr:
                p_O = psO.tile([128, F], F32)
                p_nz = psr.tile([128, F], F32, tag="rz")
                p_rev = None
                kws, t1s, svs, sns = {}, {}, {}, {}
                tap_idx = 0
                fold_ptr = 0
                folds_done = -1
                out_started = [False, False]

                for c in range(NCHUNK + 4):
                    if c < NCHUNK:
                        p_kw = pskw.tile([128, CHW], F32, tag="kw")
                        for half in range(2):
                            nc.tensor.matmul(
                                p_kw[:, half * 512:(half + 1) * 512],
                                t_kv16[:],
                                t_wall[:, c * CHW + half * 512:c * CHW + (half + 1) * 512],
                                start=True, stop=True)
                        kws[c] = p_kw
                    # noise-branch backhalf interleaved into early chunks
                    if c == 4:
                        nc.tensor.matmul(p_nz[:], t_icre16[:], t_pre[:],
                                         start=True, stop=False)
                        nc.tensor.matmul(p_nz[:], t_icim16[:], t_pim[:],
                                         start=False, stop=False)
                        nc.tensor.matmul(p_nz[:], t_nyq16[:], t_p128[:],
                                         start=False, stop=True)
                    if c == 6:
                        nc.vector.tensor_add(t_spad16[:, 126:526],
                                             t_spad16[:, 126:526], p_nz[:])
                    if c == 7:
                        p_rev = psr.tile([128, F], F32, tag="rz")
                    if c - 1 >= 0 and c - 1 < NCHUNK:
                        cj = c - 1
                        pk = kws[cj]
                        t_t1 = chk.tile([128, CHW], F32, tag="t1")
                        if cj % 5 in (1, 3):
                            nc.vector.tensor_scalar(out=t_t1[:], in0=pk[:], scalar1=MAGIC,
                                                    scalar2=None, op0=OP.add)
                        else:
                            nc.scalar.activation(t_t1[:], pk[:], AF.Identity,
                                                 bias=t_mcol[:, :], scale=1.0)
                        t1s[cj] = t_t1
                    if c - 2 >= 0 and c - 2 < NCHUNK:
                        t_sv = chk.tile([128, CHW], F32, tag="sv")
                        nc.vector.scalar_tensor_tensor(out=t_sv[:], in0=t1s.pop(c - 2)[:],
                                                       scalar=MAGIC, in1=kws.pop(c - 2)[:],
                                                       op0=OP.subtract, op1=OP.subtract)
                        svs[c - 2] = t_sv
                    if c - 3 >= 0 and c - 3 < NCHUNK:
                        t_sn = chk.tile([128, CHW], F16, tag="sn")
                        nc.scalar.activation(t_sn[:], svs.pop(c - 3)[:], AF.Sin,
                                             scale=2.0 * math.pi)
                        sns[c - 3] = t_sn
                    if c - 4 >= 0:
                        cj = c - 4
                        t_sn = sns.pop(cj)
                        for fl in range(8):
                            f = 8 * cj + fl
                            nc.tensor.matmul(p_O[:, f:f + 1],
                                             t_sn[0:NH, fl * 128:(fl + 1) * 128],
                                             t_at16[0:NH, f:f + 1], start=True, stop=True)
                        if fold_ptr < len(folds) and folds[fold_ptr] == cj:
                            fa = 8 * (folds[fold_ptr - 1] + 1) if fold_ptr > 0 else 0
                            fb = 8 * (cj + 1)
                            nc.vector.tensor_add(t_spad16[:, 126 + fa:126 + fb],
                                                 t_spad16[:, 126 + fa:126 + fb],
                                                 p_O[:, fa:fb])
                            fold_ptr += 1
                            folds_done = cj
                        # launch ready reverb taps
                        nhere = 0
                        while (tap_idx < len(taps) and nhere < TAP_BUDGET
                               and taps[tap_idx][0] <= folds_done):
                            _, ri, d = taps[tap_idx]
                            off, wdt = regions[ri]
                            nc.tensor.matmul(
                                p_rev[:, off:off + wdt],
                                t_hs16[:, d * 128:(d + 1) * 128],
                                t_spad16[:, 126 - d + off:126 - d + off + wdt],
                                start=(region_seen[ri] == 0),
                                stop=(region_seen[ri] == region_total[ri] - 1))
                            region_seen[ri] += 1
                            tap_idx += 1
                            nhere += 1
                            if region_seen[ri] == region_total[ri] and not out_started[ri]:
                                out_started[ri] = True
                                t_outr = big.tile([128, wdt], F32, tag=f"out{ri}",
                                                  name=f"out{ri}")
                                nc.scalar.copy(t_outr[:], p_rev[:, off:off + wdt])
                                nc.sync.dma_start(
                                    bass.AP(out_d, off, [[F, 128], [1, wdt]]),
                                    t_outr[:])

                # drain remaining taps
                while tap_idx < len(taps):
                    _, ri, d = taps[tap_idx]
                    off, wdt = regions[ri]
                    nc.tensor.matmul(
                        p_rev[:, off:off + wdt],
                        t_hs16[:, d * 128:(d + 1) * 128],
                        t_spad16[:, 126 - d + off:126 - d + off + wdt],
                        start=(region_seen[ri] == 0),
                        stop=(region_seen[ri] == region_total[ri] - 1))
                    region_seen[ri] += 1
                    tap_idx += 1
                    if region_seen[ri] == region_total[ri] and not out_started[ri]:
                        out_started[ri] = True
                        t_outr = big.tile([128, wdt], F32, tag=f"out{ri}",
                                          name=f"out{ri}")
                        nc.scalar.copy(t_outr[:], p_rev[:, off:off + wdt])
                        nc.sync.dma_start(
                            bass.AP(out_d, off, [[F, 128], [1, wdt]]),
                            t_outr[:])

    nc.compile()
    return nc


def _prep_inputs(inputs):
    amp = np.ascontiguousarray(np.asarray(inputs["amp_param"], np.float32))
    npr = np.ascontiguousarray(np.asarray(inputs["noise_param"], np.float32))
    pit = np.ascontiguousarray(np.asarray(inputs["pitch"], np.float32))
    noi = np.ascontiguousarray(np.asarray(inputs["noise"], np.float32))
    rvn = np.ascontiguousarray(np.asarray(inputs["reverb_noise"], np.float32))
    dec = float(np.asarray(inputs["decay"]).reshape(-1)[0])
    wet = float(np.asarray(inputs["wet"]).reshape(-1)[0])
    consts = _cache["consts"]
    in_maps = []
    for b in range(B):
        parpk = np.ascontiguousarray(
            np.concatenate([amp[b], npr[b], noi[b]], axis=1).astype(np.float32))
        pitpk = np.concatenate(
            [pit[b].reshape(F, 1),
             np.array([[dec], [wet]], np.float32)], axis=0)
        m = dict(pitchpk=np.ascontiguousarray(pitpk), parampk=parpk,
                 reverb_noise=rvn)
        m.update(consts)
        in_maps.append(m)
    return in_maps


def kernel(**inputs):
    if "nc" not in _cache:
        _cache["consts"] = _host_constants()
        _cache["nc"] = _build()
    nc = _cache["nc"]
    in_maps = _prep_inputs(inputs)
    res = run_bass_kernel_spmd(nc, in_maps, list(range(B)))
    out = np.stack([res.results[b]["out"].T.reshape(T, 1) for b in range(B)])
    return out.astype(np.float32)


if __name__ == "__main__":
    rng = np.random.default_rng(0)
    ins = dict(
        amp_param=rng.standard_normal((B, F, NH + 1)).astype(np.float32),
        noise_param=rng.standard_normal((B, F, NB)).astype(np.float32),
        pitch=(rng.random((B, F, 1), np.float32) * 440 + 60),
        noise=(rng.random((B, F, BLOCK), np.float32) * 2 - 1),
        reverb_noise=(rng.random((SR, 1), np.float32) * 2 - 1),
        decay=np.ones(1, np.float32) * 5,
        wet=np.zeros(1, np.float32),
        sampling_rate=SR, block_size=BLOCK,
    )
    o = kernel(**ins)
    print("kernel out", o.shape, o.dtype, np.abs(o).max())


def _install_ntff_hook():
    import sys as _sys
    import types as _types
    try:
        import antenv.axon_hooks  # noqa: F401
        return
    except ImportError:
        pass
    from trn_agent_boot.trn_boot import _ntff_profile_via_ctypes
    hook = _ntff_profile_via_ctypes('/opt/axon/libaxon_pjrt.so')
    mod = _types.ModuleType('antenv.axon_hooks')
    _h = {'v': hook}
    mod.get_axon_ntff_profile_hook = lambda: _h['v']
    mod.set_axon_ntff_profile_hook = lambda h: _h.update(v=h)
    _sys.modules['antenv.axon_hooks'] = mod
    import antenv
    antenv.axon_hooks = mod


def run_timed(**inputs):
    """Re-run with NTFF tracing enabled; returns max per-core exec ns or None."""
    _install_ntff_hook()
    if "nc" not in _cache:
        _cache["consts"] = _host_constants()
        _cache["nc"] = _build()
    nc = _cache["nc"]
    in_maps = _prep_inputs(inputs)
    res = run_bass_kernel_spmd(nc, in_maps, list(range(B)), trace=True)
    if res.instructions_and_trace is not None:
        _cache["insts"] = res.instructions_and_trace[0]
    return res.exec_time_ns
